# revision 4
# baseline (speedup 1.0000x reference)
"""Trainium2 Bass kernel for nn_MultiDomainPLEFENDModel (soft-MoE multi-domain FEND).

Strategy (8 NeuronCores, SPMD), v2:
  Only logits[category[b], b] is consumed, so domain-expert CNNs run on just
  that domain's samples.  Core c owns domain c: the host gathers the <=SMAX
  samples with category==c and core c runs its 6 domain experts (x2
  modalities) on them, plus the full gating/combine/MLP pipeline for those
  samples only.  The 12 shared experts per modality still need the full
  batch: the 12 row-tiles (2 experts each) x 2 modalities are split over the
  8 cores as one full-batch tile + one half-batch tile per core, results
  AllGather'ed, and each consumer core selects its own samples' columns with
  a one-hot matmul (Sel).

  Conv x / weights are stored fp8 e4m3 (scaled), accumulated fp32 in PSUM;
  the pooled/gate path stays bf16/fp32.  Final domain selection + sigmoid on
  the host.

  Per-core PE work drops from 8 full-batch expert-tiles (baseline) to
  6*SMAX/32 + 1.5 (= 2.625 at SMAX=6).
"""

import numpy as np
import ml_dtypes

import concourse.bass as bass
import concourse.tile as tile
from concourse import bacc, mybir
from concourse import bass_utils

BF16 = ml_dtypes.bfloat16
E4 = ml_dtypes.float8_e4m3
F32 = mybir.dt.float32
BF = mybir.dt.bfloat16
FP8 = mybir.dt.float8e4
ALU = mybir.AluOpType
ACTF = mybir.ActivationFunctionType

B, L, D = 32, 197, 768
LP = 200
BLP = B * LP            # 6400
HB = 16                 # half-batch for the half shared tile
HBLP = HB * LP          # 3200
WS = 100
DC = D // 128           # 6
KS = (1, 2, 3, 5, 10)
FK = 64
GATE_E = 18
NCORES = 8
NSLOT = 8               # 6 domain + full-shared + half-shared
MODN = {0: "t", 1: "i"}

S_X = 16.0              # fp8 scale for x
S_W = 2048.0            # fp8 scale for conv weights / aw
S_FEAT = S_X * S_W


def shared_assign(c):
    """(modality, full_tile, half_tile, half_idx) of core c's shared slots."""
    msh = 0 if c < 4 else 1
    q = c % 4
    return msh, q, 4 + q // 2, q % 2


# ---------------------------------------------------------------------------
# Bass module
# ---------------------------------------------------------------------------

def build_nc(smax=6, reps=1, no_cc=False):
    assert smax % 2 == 0 and 2 <= smax <= 32
    wnd = smax * LP // WS

    nc = bacc.Bacc(
        "TRN2",
        target_bir_lowering=False,
        debug=False,
        enable_asserts=False,
        num_devices=NCORES,
    )

    di = {}

    def inp(name, shape, dt):
        di[name] = nc.dram_tensor(name, list(shape), dt, kind="ExternalInput")

    for k in KS:
        inp(f"w_k{k}", (NSLOT, 128, k, DC, 128), FP8)
    inp("cbias", (128, NSLOT, 5), F32)
    inp("xt_sh", (DC, 128, BLP), FP8)
    inp("xt_hf", (DC, 128, HBLP), FP8)
    for m in (0, 1):
        inp(f"xd_{MODN[m]}", (DC, 128, smax * LP), FP8)
        inp(f"xnd_{MODN[m]}", (wnd, WS, D), BF)
    inp("b_ind_d", (WS, wnd, smax), BF)
    inp("mask2d", (smax, 2, LP), F32)
    inp("aw", (128, DC, 2), FP8)
    inp("dom_embT", (128, DC, smax), F32)
    inp("sel32", (32, smax), BF)
    inp("selh", (HB, 2, smax), BF)
    inp("gw1", (2, 12, 128, DC, 128), F32)
    inp("gb1", (smax, 2, D), F32)
    inp("gw2", (2, 128, DC, GATE_E), F32)
    inp("gb2", (smax, 2, GATE_E), F32)
    inp("cw1", (2, 128, 3, 3, 128), F32)
    inp("cb1", (2, 128, 3), F32)
    inp("cw2", (2, 128, 3), F32)
    inp("cb2", (2, 1, 1), F32)
    inp("ident128", (128, 128), F32)

    out_dram = nc.dram_tensor("logits", [2, 1, smax], F32, kind="ExternalOutput")

    ag_in = nc.dram_tensor("agin", [128, 5, B + HB], F32, kind="Internal")
    ag_out = nc.dram_tensor("agout", [NCORES, 128, 5, B + HB], F32,
                            kind="Internal", addr_space="Shared")

    with tile.TileContext(nc) as tc:
        _program(nc, tc, di, out_dram, ag_in, ag_out, smax, wnd, reps, no_cc)

    nc.compile()
    return nc


def _program(nc, tc, di, out_dram, ag_in, ag_out, smax, wnd, reps, no_cc):
    counter = [0]

    def nm(base):
        counter[0] += 1
        return f"{base}{counter[0]}"

    import contextlib
    with contextlib.ExitStack() as ctx:
        ep = ctx.enter_context
        xt_pool = ep(tc.tile_pool(name="xt", bufs=1))
        wk_pool = ep(tc.tile_pool(name="wk", bufs=2))
        xn_pool = ep(tc.tile_pool(name="xn", bufs=4))
        feat_pool = ep(tc.tile_pool(name="feat", bufs=1))
        sh_pool = ep(tc.tile_pool(name="sh", bufs=1))
        shT_pool = ep(tc.tile_pool(name="shT", bufs=3))
        featb_pool = ep(tc.tile_pool(name="featb", bufs=3))
        small = ep(tc.tile_pool(name="small", bufs=2))
        small1 = ep(tc.tile_pool(name="small1", bufs=1))
        const_pool = ep(tc.tile_pool(name="const", bufs=1))
        gw1_pool = ep(tc.tile_pool(name="gw1p", bufs=2))
        comb_pool = ep(tc.tile_pool(name="comb", bufs=2))
        combt_pool = ep(tc.tile_pool(name="combt", bufs=6))
        psum_conv = ep(tc.tile_pool(name="pconv", bufs=6, space="PSUM"))
        psum_misc = ep(tc.tile_pool(name="pmisc", bufs=2, space="PSUM"))

        # ---- resident constants ----
        def cget(name, shape, dt, src=None):
            t = const_pool.tile(shape, dt, tag=name)
            if src is not None:
                nc.sync.dma_start(t[:], src)
            return t

        cbias = cget("cbias", [128, NSLOT, 5], F32, di["cbias"][:])
        b_ind = cget("bind", [WS, wnd, smax], BF, di["b_ind_d"][:])
        aw = cget("aw", [128, DC, 2], FP8, di["aw"][:])
        ident = cget("ident", [128, 128], F32, di["ident128"][:])
        dom_embT = cget("domT", [128, DC, smax], F32, di["dom_embT"][:])
        sel32 = cget("sel32", [32, smax], BF, di["sel32"][:])
        selh = cget("selh", [HB, 2, smax], BF, di["selh"][:])
        mask2 = cget("mask2", [smax, 2, LP], F32, di["mask2d"][:])
        gb1 = cget("gb1", [smax, 2, D], F32, di["gb1"][:])
        gw2 = cget("gw2", [128, 2, DC, GATE_E], F32)
        gb2 = cget("gb2", [smax, 2, GATE_E], F32, di["gb2"][:])
        cw1 = cget("cw1", [128, 2, 3, 3, 128], F32)
        cb1 = cget("cb1", [128, 2, 3], F32)
        cw2 = cget("cw2", [128, 2, 3], F32)
        cb2 = cget("cb2", [1, 2, 1], F32)
        for m in (0, 1):
            nc.sync.dma_start(gw2[:, m, :, :], di["gw2"][m])
            nc.sync.dma_start(cw1[:, m, :, :, :], di["cw1"][m])
            nc.sync.dma_start(cb1[:, m, :], di["cb1"][m])
            nc.sync.dma_start(cw2[:, m, :], di["cw2"][m])
            nc.sync.dma_start(cb2[0:1, m, :], di["cb2"][m])

        # feat tiles: slots 0-5 domain [128,5,smax]; 6 full [128,5,32];
        # 7 half [128,5,16]
        fshape = {s: smax for s in range(6)}
        fshape[6] = B
        fshape[7] = HB
        feat = {s: feat_pool.tile([128, 5, fshape[s]], F32, tag=f"feat{s}",
                                  name=f"feat{s}")
                for s in range(NSLOT)}
        sh_sb = sh_pool.tile([128, NCORES, 5, B + HB], F32, tag="shsb")
        gate_sb = {}

        def conv_slot(s, xv, nb):
            """xv: [128, DC, nb, LP] AP; nb samples."""
            for ki, k in enumerate(KS):
                lo = L - k + 1
                wk = wk_pool.tile([128, k, DC, 128], FP8, tag="wk")
                nc.sync.dma_start(wk[:], di[f"w_k{k}"][s])
                for bb in range(nb // 2):
                    pt = psum_conv.tile([128, 2, lo], F32, tag="conv")
                    n = 0
                    for dcc in range(DC):
                        for j in range(k):
                            nc.tensor.matmul(
                                pt[:],
                                wk[:, j, dcc, :],
                                xv[:, dcc, 2 * bb:2 * bb + 2, j:j + lo],
                                start=(n == 0), stop=(n == DC * k - 1))
                            n += 1
                    nc.vector.reduce_max(
                        feat[s][:, ki, 2 * bb:2 * bb + 2], pt[:],
                        axis=mybir.AxisListType.X)
            for ki in range(len(KS)):
                nc.vector.tensor_scalar_add(
                    feat[s][:, ki, :], feat[s][:, ki, :],
                    cbias[:, s, ki:ki + 1])

        def scores_pool_gates(mod, xd):
            # ---- scores for the domain samples (fp8, scaled by S_FEAT) ----
            s2 = small.tile([smax, LP], F32, tag="s2")
            for sl in range(smax // 2):
                spt = psum_misc.tile([1, 2 * LP], F32, tag="misc",
                                     name=nm("spt"))
                for dcc in range(DC):
                    nc.tensor.matmul(
                        spt[:], aw[:, dcc, mod:mod + 1],
                        xd[:, dcc, sl * 2 * LP:(sl + 1) * 2 * LP],
                        start=(dcc == 0), stop=(dcc == DC - 1))
                scp = small.tile([1, 2 * LP], F32, tag="scp", name=nm("scp"))
                nc.scalar.activation(scp[:], spt[:], ACTF.Identity,
                                     scale=1.0 / S_FEAT)
                nc.sync.dma_start(s2[2 * sl:2 * sl + 2, :], scp[:])
            # ---- masked softmax over l ----
            nc.vector.scalar_tensor_tensor(
                out=s2[:], in0=s2[:], scalar=1e9, in1=mask2[:, mod, :],
                op0=ALU.add, op1=ALU.mult)
            nc.vector.tensor_scalar_sub(s2[:], s2[:], 1e9)
            mx = small.tile([smax, 1], F32, tag="mx")
            nc.vector.reduce_max(mx[:], s2[:], axis=mybir.AxisListType.X)
            nc.vector.tensor_scalar_sub(s2[:], s2[:], mx[:, 0:1])
            sm = small.tile([smax, 1], F32, tag="sm")
            nc.scalar.activation(s2[:], s2[:], ACTF.Exp, accum_out=sm[:])
            rd = small.tile([smax, 1], F32, tag="rd")
            nc.vector.reciprocal(rd[:], sm[:])
            nc.vector.tensor_scalar_mul(s2[:], s2[:], rd[:, 0:1])
            # ---- p -> pr [WS, wnd] ----
            pT = small.tile([wnd, WS], F32, tag="pT")
            nc.sync.dma_start(pT[:], s2[:])
            tp2 = psum_misc.tile([WS, wnd], F32, tag="misc")
            nc.tensor.transpose(tp2[:], pT[:], ident[0:wnd, 0:wnd])
            pr = small.tile([WS, wnd], F32, tag="pr")
            nc.scalar.copy(pr[:], tp2[:])
            # ---- P = b_ind * pr ----
            P = small1.tile([WS, wnd, smax], BF, tag="P")
            for ch in range(wnd):
                nc.vector.tensor_scalar_mul(
                    P[:, ch, :], b_ind[:, ch, :], pr[:, ch:ch + 1])
            # ---- pooled [smax, 768] (samples on partitions) ----
            gin = small1.tile([128, 12, smax], F32, tag="ginT")
            nc.scalar.copy(gin[:, 6:12, :], dom_embT[:])
            pba = psum_misc.tile([smax, 512], F32, tag="misc", name=nm("pba"))
            pbb = psum_misc.tile([smax, D - 512], F32, tag="misc", name=nm("pbb"))
            for ch in range(wnd):
                xn = xn_pool.tile([WS, D], BF, tag="xn")
                nc.sync.dma_start(xn[:], di[f"xnd_{MODN[mod]}"][ch])
                nc.tensor.matmul(pba[:], P[:, ch, :], xn[:, 0:512],
                                 start=(ch == 0), stop=(ch == wnd - 1))
                nc.tensor.matmul(pbb[:], P[:, ch, :], xn[:, 512:D],
                                 start=(ch == 0), stop=(ch == wnd - 1))
            pb_sb = small1.tile([smax, D], F32, tag="pbsb")
            nc.scalar.copy(pb_sb[:, 0:512], pba[:])
            nc.scalar.copy(pb_sb[:, 512:D], pbb[:])
            for dcc in range(DC):
                tpp = psum_misc.tile([128, smax], F32, tag="misc", name=nm("tpp"))
                nc.tensor.transpose(
                    tpp[:], pb_sb[:, dcc * 128:(dcc + 1) * 128],
                    ident[0:smax, 0:smax])
                nc.scalar.copy(gin[:, dcc, :], tpp[:])
            # ---- gate MLP ----
            hba = psum_misc.tile([smax, 512], F32, tag="misc", name=nm("hba"))
            hbb = psum_misc.tile([smax, D - 512], F32, tag="misc", name=nm("hbb"))
            for ic in range(12):
                g1 = gw1_pool.tile([128, D], F32, tag="gw1c")
                nc.sync.dma_start(g1[:], di["gw1"][mod, ic])
                nc.tensor.matmul(hba[:], gin[:, ic, :], g1[:, 0:512],
                                 start=(ic == 0), stop=(ic == 11))
                nc.tensor.matmul(hbb[:], gin[:, ic, :], g1[:, 512:D],
                                 start=(ic == 0), stop=(ic == 11))
            h_sb = small1.tile([smax, D], F32, tag="hsb")
            nc.vector.tensor_tensor(
                out=h_sb[:, 0:512], in0=hba[:], in1=gb1[:, mod, 0:512],
                op=ALU.add)
            nc.vector.tensor_tensor(
                out=h_sb[:, 512:D], in0=hbb[:], in1=gb1[:, mod, 512:D],
                op=ALU.add)
            hsg = small1.tile([smax, D], F32, tag="hsg")
            nc.scalar.activation(hsg[:], h_sb[:], ACTF.Sigmoid)
            nc.vector.tensor_tensor(
                out=h_sb[:], in0=h_sb[:], in1=hsg[:], op=ALU.mult)
            hT = small1.tile([128, DC, smax], F32, tag="hT")
            for oc in range(DC):
                tph = psum_misc.tile([128, smax], F32, tag="misc", name=nm("tph"))
                nc.tensor.transpose(
                    tph[:], h_sb[:, oc * 128:(oc + 1) * 128],
                    ident[0:smax, 0:smax])
                nc.scalar.copy(hT[:, oc, :], tph[:])
            # ---- gate logits + softmax ----
            gl_ps = psum_misc.tile([smax, GATE_E], F32, tag="misc")
            for oc in range(DC):
                nc.tensor.matmul(
                    gl_ps[:], hT[:, oc, :], gw2[:, mod, oc, :],
                    start=(oc == 0), stop=(oc == DC - 1))
            gate = small.tile([smax, GATE_E], F32, tag="gate")
            nc.vector.tensor_tensor(
                out=gate[:], in0=gl_ps[:], in1=gb2[:, mod, :], op=ALU.add)
            gmx = small.tile([smax, 1], F32, tag="gmx")
            nc.vector.reduce_max(gmx[:], gate[:], axis=mybir.AxisListType.X)
            nc.vector.tensor_scalar_sub(gate[:], gate[:], gmx[:, 0:1])
            gsm = small.tile([smax, 1], F32, tag="gsm")
            nc.scalar.activation(gate[:], gate[:], ACTF.Exp, accum_out=gsm[:])
            grd = small.tile([smax, 1], F32, tag="grd")
            nc.vector.reciprocal(grd[:], gsm[:])
            nc.vector.tensor_scalar_mul(gate[:], gate[:], grd[:, 0:1])
            return gate

        def make_featb_local(slot):
            fb = featb_pool.tile([smax, 5, 128], F32, tag="featb",
                                 name=nm("fbl"))
            for ki in range(5):
                tpf = psum_misc.tile([smax, 128], F32, tag="misc", name=nm("tpf"))
                nc.tensor.transpose(tpf[:], feat[slot][:, ki, :], ident[:])
                nc.scalar.copy(fb[:, ki, :], tpf[:])
            return fb

        def make_featb_shared(mod, t):
            fb = featb_pool.tile([smax, 5, 128], F32, tag="featb",
                                 name=nm("fbs"))
            for ki in range(5):
                selp = psum_misc.tile([smax, 128], F32, tag="misc",
                                      name=nm("selp"))
                if t < 4:
                    rank = 4 * mod + t
                    trs = psum_misc.tile([B, 128], F32, tag="misc",
                                         name=nm("trs"))
                    nc.tensor.transpose(
                        trs[:], sh_sb[:, rank, ki, 0:B], ident[:])
                    shT = shT_pool.tile([B, 128], BF, tag="shT", name=nm("shT"))
                    nc.scalar.copy(shT[:], trs[:])
                    nc.tensor.matmul(selp[:], sel32[:], shT[:],
                                     start=True, stop=True)
                else:
                    ra = 4 * mod + 2 * (t - 4)
                    for h in (0, 1):
                        trs = psum_misc.tile([HB, 128], F32, tag="misc",
                                             name=nm("trs"))
                        nc.tensor.transpose(
                            trs[:], sh_sb[:, ra + h, ki, B:B + HB], ident[:])
                        shT = shT_pool.tile([HB, 128], BF, tag="shTh",
                                            name=nm("shTh"))
                        nc.scalar.copy(shT[:], trs[:])
                        nc.tensor.matmul(selp[:], selh[:, h, :], shT[:],
                                         start=(h == 0), stop=(h == 1),
                                         skip_group_check=True)
                nc.scalar.copy(fb[:, ki, :], selp[:])
            return fb

        def combine_mlp(mod, gate):
            comb_b = comb_pool.tile([smax, 3 * 128], F32, tag="combb",
                                    name=nm("combb"))
            nc.vector.memset(comb_b[:], 0.0)

            def accum(fb, e_base):
                for eloc in (0, 1):
                    e = e_base + eloc
                    for ki in range(5):
                        cs = comb_b[:, ki * 64:(ki + 1) * 64]
                        nc.vector.scalar_tensor_tensor(
                            out=cs, in0=fb[:, ki, 64 * eloc:64 * eloc + 64],
                            scalar=gate[:, e:e + 1], in1=cs,
                            op0=ALU.mult, op1=ALU.add)

            for si in range(3):
                accum(make_featb_local(3 * mod + si), 2 * si)
            for t in range(6):
                accum(make_featb_shared(mod, t), 6 + 2 * t)

            combT = [combt_pool.tile([128, smax], F32, tag="combT",
                                     name=nm("combT")) for _ in range(3)]
            for ck in range(3):
                tpc = psum_misc.tile([128, smax], F32, tag="misc", name=nm("tpc"))
                nc.tensor.transpose(
                    tpc[:], comb_b[:, ck * 128:(ck + 1) * 128],
                    ident[0:smax, 0:smax])
                nc.scalar.copy(combT[ck][:], tpc[:])
            hhT = small.tile([128, 3, smax], F32, tag="hhT")
            for mc in range(3):
                hh_ps = psum_misc.tile([128, smax], F32, tag="misc",
                                       name=nm("hhps"))
                for kc in range(3):
                    nc.tensor.matmul(
                        hh_ps[:], cw1[:, mod, kc, mc, :], combT[kc][:],
                        start=(kc == 0), stop=(kc == 2))
                nc.scalar.activation(
                    hhT[:, mc, :], hh_ps[:], ACTF.Relu,
                    bias=cb1[:, mod, mc:mc + 1])
            lg_ps = psum_misc.tile([1, smax], F32, tag="misc")
            for kc in range(3):
                nc.tensor.matmul(
                    lg_ps[:], cw2[:, mod, kc:kc + 1], hhT[:, kc, :],
                    start=(kc == 0), stop=(kc == 2))
            lg = small.tile([1, smax], F32, tag="lg")
            nc.scalar.activation(lg[:], lg_ps[:], ACTF.Identity,
                                 bias=cb2[0:1, mod, :])
            nc.sync.dma_start(out_dram[mod], lg[:])

        # ================= main program =================
        for rep in range(reps):
            # shared-modality full batch + half batch
            xt_sh = xt_pool.tile([128, DC, BLP], FP8, tag="xtsh")
            xt_hf = xt_pool.tile([128, DC, HBLP], FP8, tag="xthf")
            for dcc in range(DC):
                nc.sync.dma_start(xt_sh[:, dcc, :], di["xt_sh"][dcc])
                nc.sync.dma_start(xt_hf[:, dcc, :], di["xt_hf"][dcc])
            xd = {}
            for m in (0, 1):
                xd[m] = xt_pool.tile([128, DC, smax * LP], FP8, tag=f"xd{m}",
                                     name=f"xd{m}")
                for dcc in range(DC):
                    nc.sync.dma_start(xd[m][:, dcc, :], di[f"xd_{MODN[m]}"][dcc])

            conv_slot(6, xt_sh[:].rearrange("p c (b l) -> p c b l", b=B), B)
            conv_slot(7, xt_hf[:].rearrange("p c (b l) -> p c b l", b=HB), HB)
            nc.sync.dma_start(ag_in[:, :, 0:B], feat[6][:])
            nc.sync.dma_start(ag_in[:, :, B:B + HB], feat[7][:])
            if no_cc:
                for r in range(NCORES):
                    nc.sync.dma_start(ag_out[r], ag_in[:])
            else:
                nc.gpsimd.collective_compute(
                    "AllGather", ALU.bypass,
                    replica_groups=[list(range(NCORES))],
                    ins=[ag_in[:].opt()],
                    outs=[ag_out[:].opt()])

            for mod in (0, 1):
                gate_sb[mod] = scores_pool_gates(mod, xd[mod][:])
                xv = xd[mod][:].rearrange("p c (b l) -> p c b l", b=smax)
                for si in range(3):
                    conv_slot(3 * mod + si, xv, smax)

            for r in range(NCORES):
                nc.sync.dma_start(sh_sb[:, r, :, :], ag_out[r])
            for mod in (0, 1):
                combine_mlp(mod, gate_sb[mod])


# ---------------------------------------------------------------------------
# Host-side preparation
# ---------------------------------------------------------------------------

def f32(x):
    return np.ascontiguousarray(np.asarray(x, np.float32))


def q8(x, scale):
    return np.clip(np.asarray(x, np.float32) * scale, -240, 240).astype(E4)


def host_prep(inputs, smax):
    wnd = smax * LP // WS
    xs = {0: f32(inputs["text_feature"]), 1: f32(inputs["image_feature"])}
    cat = np.asarray(inputs["category"], np.int64)
    MODF = {0: "text", 1: "image"}

    perms, cnts = [], []
    for c in range(NCORES):
        idx = np.where(cat == c)[0]
        cnts.append(len(idx))
        perms.append(np.concatenate(
            [idx, np.zeros(smax - len(idx), np.int64)]))

    flat, xt8 = {}, {}
    for m in (0, 1):
        xp = np.zeros((B, LP, D), np.float32)
        xp[:, :L, :] = xs[m]
        flat[m] = xp.reshape(BLP, D)
        xt8[m] = np.ascontiguousarray(
            np.clip(flat[m].T * S_X, -240, 240).reshape(DC, 128, BLP)
        ).astype(E4)

    # b_ind_d: constant structure (flat dom index r = ch*WS+row ->
    # sample r // LP, pos r % LP, valid when pos < L)
    r = np.arange(smax * LP)
    bi = np.zeros((smax * LP, smax), np.float32)
    valid = (r % LP) < L
    bi[valid, (r[valid] // LP)] = 1.0
    b_ind_d = np.ascontiguousarray(
        bi.reshape(wnd, WS, smax).transpose(1, 0, 2)).astype(BF16)

    awp = np.zeros((128, DC, 2), np.float32)
    for m in (0, 1):
        awp[:, :, m] = f32(inputs[f"{MODF[m]}_aw"]).reshape(DC, 128).T
    aw8 = q8(awp, S_W)

    masks = f32(inputs["masks"])
    dom_emb = f32(inputs["domain_emb"])

    in_maps = []
    for c in range(NCORES):
        msh, ft, ht, half = shared_assign(c)
        perm = perms[c]
        d = {"xt_sh": xt8[msh],
             "xt_hf": np.ascontiguousarray(
                 xt8[msh][:, :, half * HBLP:(half + 1) * HBLP]),
             "b_ind_d": b_ind_d, "aw": aw8,
             "ident128": np.eye(128, dtype=np.float32)}

        for m in (0, 1):
            fd = flat[m].reshape(B, LP, D)[perm].reshape(smax * LP, D)
            d[f"xd_{MODN[m]}"] = np.ascontiguousarray(
                np.clip(fd.T * S_X, -240, 240).reshape(DC, 128, smax * LP)
            ).astype(E4)
            d[f"xnd_{MODN[m]}"] = np.ascontiguousarray(
                fd.reshape(wnd, WS, D)).astype(BF16)

        m2 = np.zeros((smax, 2, LP), np.float32)
        m2[:, 0, :L] = (masks[perm] > 0).astype(np.float32)
        m2[:, 1, :L] = 1.0
        d["mask2d"] = m2

        d["dom_embT"] = np.ascontiguousarray(np.repeat(
            dom_emb[c].reshape(DC, 128).T[:, :, None], smax, axis=2))

        sel = np.zeros((B, smax), np.float32)
        sel[perm, np.arange(smax)] = 1.0
        d["sel32"] = sel.astype(BF16)
        selh = np.zeros((HB, 2, smax), np.float32)
        for s in range(smax):
            p = perm[s]
            selh[p % HB, p // HB, s] = 1.0
        d["selh"] = selh.astype(BF16)

        # conv weights: slots 0-2 text domain pairs, 3-5 image domain pairs,
        # 6 full shared tile, 7 half shared tile
        def slot_experts(s):
            if s < 3:
                return 0, (6 * c + 2 * s, 6 * c + 2 * s + 1)
            if s < 6:
                return 1, (6 * c + 2 * (s - 3), 6 * c + 2 * (s - 3) + 1)
            t = ft if s == 6 else ht
            return msh, (48 + 2 * t, 49 + 2 * t)

        for k in KS:
            wk = np.zeros((NSLOT, 128, k, DC, 128), np.float32)
            for s in range(NSLOT):
                mod, es = slot_experts(s)
                wsrc = f32(inputs[f"{MODF[mod]}_cw_k{k}"])
                for el, e in enumerate(es):
                    w_e = wsrc[e]       # [FK, D, k]
                    wt = w_e.transpose(1, 2, 0).reshape(
                        DC, 128, k, FK).transpose(1, 2, 0, 3)
                    wk[s, :, :, :, el * 64:(el + 1) * 64] = wt
            d[f"w_k{k}"] = q8(wk, S_W)
        cb = np.zeros((128, NSLOT, len(KS)), np.float32)
        for s in range(NSLOT):
            mod, es = slot_experts(s)
            cbs = f32(inputs[f"{MODF[mod]}_cb"])
            for el, e in enumerate(es):
                cb[el * 64:(el + 1) * 64, s, :] = cbs[:, e, :].T
        d["cbias"] = cb * S_FEAT

        d["gw1"] = np.stack([f32(inputs[f"{MODF[m]}_gw1"])[c] for m in (0, 1)]
                            ).reshape(2, 12, 128, DC, 128).copy()
        gb1 = np.stack([f32(inputs[f"{MODF[m]}_gb1"])[c] for m in (0, 1)])
        d["gb1"] = np.ascontiguousarray(
            np.repeat(gb1[None, :, :], smax, axis=0))
        d["gw2"] = np.ascontiguousarray(
            np.stack([f32(inputs[f"{MODF[m]}_gw2"])[c] for m in (0, 1)]
                     ).reshape(2, DC, 128, GATE_E).transpose(0, 2, 1, 3))
        gb2 = np.stack([f32(inputs[f"{MODF[m]}_gb2"])[c] for m in (0, 1)])
        d["gb2"] = np.ascontiguousarray(
            np.repeat(gb2[None, :, :], smax, axis=0))
        cw1 = np.stack([f32(inputs[f"{MODF[m]}_cw1"])[c] for m in (0, 1)])
        cw1p = np.zeros((2, 384, 384), np.float32)
        cw1p[:, :320, :] = cw1 / S_FEAT
        d["cw1"] = np.ascontiguousarray(
            cw1p.reshape(2, 3, 128, 3, 128).transpose(0, 2, 1, 3, 4))
        d["cb1"] = np.ascontiguousarray(
            np.stack([f32(inputs[f"{MODF[m]}_cb1"])[c] for m in (0, 1)]
                     ).reshape(2, 3, 128).transpose(0, 2, 1))
        cw2 = np.stack([f32(inputs[f"{MODF[m]}_cw2"])[c] for m in (0, 1)])
        d["cw2"] = np.ascontiguousarray(
            cw2.reshape(2, 3, 128).transpose(0, 2, 1))
        d["cb2"] = np.stack([f32(inputs[f"{MODF[m]}_cb2"])[c] for m in (0, 1)]
                            ).reshape(2, 1, 1).copy()
        in_maps.append(d)
    return in_maps, cat, perms, cnts


_NC_CACHE = {}


def _get_nc(smax=6, reps=1):
    key = (smax, reps)
    if key not in _NC_CACHE:
        _NC_CACHE[key] = build_nc(smax=smax, reps=reps)
    return _NC_CACHE[key]


def pick_smax(cat):
    mx = int(np.bincount(np.asarray(cat, np.int64), minlength=NCORES).max())
    return max(6, mx + (mx % 2))


def kernel(**inputs):
    cat = np.asarray(inputs["category"], np.int64)
    smax = pick_smax(cat)
    nc = _get_nc(smax=smax)
    in_maps, cat, perms, cnts = host_prep(inputs, smax)
    res = bass_utils.run_bass_kernel_spmd(
        nc, in_maps, core_ids=list(range(NCORES)))
    t_pred = np.zeros(B, np.float32)
    i_pred = np.zeros(B, np.float32)
    for c in range(NCORES):
        lg = res.results[c]["logits"]        # [2, 1, smax]
        for s in range(cnts[c]):
            b = perms[c][s]
            t_pred[b] = 1.0 / (1.0 + np.exp(-np.float64(lg[0, 0, s])))
            i_pred[b] = 1.0 / (1.0 + np.exp(-np.float64(lg[1, 0, s])))
    return t_pred, i_pred


if __name__ == "__main__":
    import time
    t0 = time.time()
    build_nc()
    print(f"build+compile: {time.time()-t0:.1f}s")
